# revision 2
# baseline (speedup 1.0000x reference)
"""Deformable-encoder block, data-parallel across 8 NeuronCores.

Sharding: (batch b in 0..3) x (query half) -> 8 shards. Each shard gets its
3200 queries of batch b plus the FULL value[b] grid (the deformable gather can
sample anywhere in the 80x80 grid). Weights are replicated. Outputs are
concatenated back on host. All FLOPs run on device via one pmap'd program.
"""

import functools

import jax
import jax.numpy as jnp
import numpy as np

HEADS = 8
POINTS = 4
EPS = 1e-5

# Problem shape (hardcoded per contract).
N, LQ, C, H, W = 4, 6400, 256, 80, 80
NSH = 8                       # shards / cores
QSH = LQ // 2                 # queries per shard


def _layernorm(x, w, b):
    m = jnp.mean(x, axis=-1, keepdims=True)
    v = jnp.var(x, axis=-1, keepdims=True)
    return (x - m) / jnp.sqrt(v + EPS) * w + b


def _shard_fn(x, ref, value, ln1_w, ln1_b, ln2_w, ln2_b, Wv, bv, Woff, boff,
              Wa, ba, Wout, bout, W1, b1, W2, b2):
    # x: (1, QSH, C); value: (1, H*W, C); ref: (1, QSH, 1, 2)
    xb = x
    a_in = _layernorm(xb, ln1_w, ln1_b)
    n, Lq, c = a_in.shape
    hd = c // HEADS

    v = (value @ Wv + bv).reshape(n, H * W, HEADS, hd).transpose(0, 2, 1, 3)
    off = (a_in @ Woff + boff).reshape(n, Lq, HEADS, POINTS, 2)
    attw = jax.nn.softmax((a_in @ Wa + ba).reshape(n, Lq, HEADS, POINTS), axis=-1)
    offset_normalizer = jnp.array([W, H], dtype=x.dtype)
    loc = ref[:, :, None, :, :] + off / offset_normalizer
    gx = loc[..., 0] * W - 0.5
    gy = loc[..., 1] * H - 0.5
    x0 = jnp.floor(gx)
    y0 = jnp.floor(gy)
    wx = gx - x0
    wy = gy - y0
    x0i = x0.astype(jnp.int32)
    y0i = y0.astype(jnp.int32)

    # Quad table: one row per base position holding all 4 bilinear corners
    # (base, base+1, base+W, base+W+1), zero-padded 81 rows front so clamped
    # bases (y0 in [-1,H-1], x0 in [-1,W-1] -> flat in [-81, HW-1]) stay in
    # range. Out-of-grid corners contribute with weight 0 below, matching the
    # reference's zero padding exactly.
    PAD = 81
    L = H * W + PAD + 1  # max shifted idx = (H*W - 1) + PAD
    vp = jnp.pad(v, ((0, 0), (0, 0), (PAD, PAD + 2), (0, 0)))
    quad = jnp.concatenate(
        [vp[:, :, o:o + L] for o in (0, 1, W, W + 1)], axis=3
    )  # (n, HEADS, L, 4*hd)

    base = (jnp.clip(y0i, -1, H - 1) * W + jnp.clip(x0i, -1, W - 1) + PAD)
    idx = base.transpose(0, 2, 1, 3).reshape(n, HEADS, Lq * POINTS)
    g = jnp.take_along_axis(quad, idx[..., None], axis=2)
    samp = g.reshape(n, HEADS, Lq, POINTS, 4, hd)

    vx0 = ((x0i >= 0) & (x0i < W)).astype(x.dtype)
    vx1 = ((x0i + 1 >= 0) & (x0i + 1 < W)).astype(x.dtype)
    vy0 = ((y0i >= 0) & (y0i < H)).astype(x.dtype)
    vy1 = ((y0i + 1 >= 0) & (y0i + 1 < H)).astype(x.dtype)
    u0 = (1 - wx) * vx0
    u1 = wx * vx1
    t0 = (1 - wy) * vy0
    t1 = wy * vy1
    cw = jnp.stack([u0 * t0, u1 * t0, u0 * t1, u1 * t1], axis=-1)  # (n,Lq,HEADS,P,4)
    cw = cw * attw[..., None]
    out = jnp.einsum('nhlpcd,nlhpc->nlhd', samp, cw).reshape(n, Lq, c)
    a = out @ Wout + bout

    xr = xb + a
    h = _layernorm(xr, ln2_w, ln2_b)
    h = jax.nn.gelu(h @ W1 + b1, approximate=False)
    return xr + (h @ W2 + b2)


@functools.partial(jax.pmap, axis_name='i',
                   in_axes=(0, 0, 0) + (None,) * 16)
def _pmapped(x, ref, value, *weights):
    return _shard_fn(x, ref, value, *weights)


def kernel(**inputs) -> np.ndarray:
    x = np.asarray(inputs['x'], np.float32)
    ref = np.asarray(inputs['ref'], np.float32)
    value = np.asarray(inputs['value'], np.float32)
    wnames = ['ln1_w', 'ln1_b', 'ln2_w', 'ln2_b', 'Wv', 'bv', 'Woff', 'boff',
              'Wa', 'ba', 'Wout', 'bout', 'W1', 'b1', 'W2', 'b2']
    weights = [np.asarray(inputs[k], np.float32) for k in wnames]

    # Build shard stacks: shard s = (batch s//2, query half s%2).
    xs = np.stack([x[s // 2, (s % 2) * QSH:(s % 2 + 1) * QSH][None] for s in range(NSH)])
    refs = np.stack([ref[s // 2, (s % 2) * QSH:(s % 2 + 1) * QSH][None] for s in range(NSH)])
    vals = np.stack([value[s // 2][None] for s in range(NSH)])

    out = np.asarray(_pmapped(xs, refs, vals, *weights))  # (8, 1, QSH, C)
    res = np.empty((N, LQ, C), np.float32)
    for s in range(NSH):
        res[s // 2, (s % 2) * QSH:(s % 2 + 1) * QSH] = out[s, 0]
    return res


# revision 3
# speedup vs baseline: 42.5004x; 42.5004x over previous
"""Deformable-encoder block, data-parallel across 8 NeuronCores.

Sharding: (batch b in 0..3) x (query half) -> 8 shards. Each shard gets its
3200 queries of batch b plus the FULL value[b] grid (the deformable gather can
sample anywhere in the 80x80 grid). Weights are replicated. Outputs are
concatenated back on host. All FLOPs run on device via one pmap'd program.
"""

import functools

import jax
import jax.numpy as jnp
import numpy as np

HEADS = 8
POINTS = 4
EPS = 1e-5

# Problem shape (hardcoded per contract).
N, LQ, C, H, W = 4, 6400, 256, 80, 80
NSH = 8                       # shards / cores
QSH = LQ // 2                 # queries per shard


def _layernorm(x, w, b):
    m = jnp.mean(x, axis=-1, keepdims=True)
    v = jnp.var(x, axis=-1, keepdims=True)
    return (x - m) / jnp.sqrt(v + EPS) * w + b


def _shard_fn(x, ref, value, ln1_w, ln1_b, ln2_w, ln2_b, Wv, bv, Woff, boff,
              Wa, ba, Wout, bout, W1, b1, W2, b2):
    # x: (1, QSH, C); value: (1, H*W, C); ref: (1, QSH, 1, 2)
    xb = x
    a_in = _layernorm(xb, ln1_w, ln1_b)
    n, Lq, c = a_in.shape
    hd = c // HEADS

    v = (value @ Wv + bv).reshape(n, H * W, HEADS, hd).transpose(0, 2, 1, 3)
    off = (a_in @ Woff + boff).reshape(n, Lq, HEADS, POINTS, 2)
    attw = jax.nn.softmax((a_in @ Wa + ba).reshape(n, Lq, HEADS, POINTS), axis=-1)
    offset_normalizer = jnp.array([W, H], dtype=x.dtype)
    loc = ref[:, :, None, :, :] + off / offset_normalizer
    gx = loc[..., 0] * W - 0.5
    gy = loc[..., 1] * H - 0.5
    x0 = jnp.floor(gx)
    y0 = jnp.floor(gy)
    wx = gx - x0
    wy = gy - y0
    x0i = x0.astype(jnp.int32)
    y0i = y0.astype(jnp.int32)

    # Quad table: one row per base position holding all 4 bilinear corners
    # (base, base+1, base+W, base+W+1), zero-padded 81 rows front so clamped
    # bases (y0 in [-1,H-1], x0 in [-1,W-1] -> flat in [-81, HW-1]) stay in
    # range. Out-of-grid corners contribute with weight 0 below, matching the
    # reference's zero padding exactly.
    PAD = 81
    L = H * W + PAD + 1  # max shifted idx = (H*W - 1) + PAD
    vp = jnp.pad(v, ((0, 0), (0, 0), (PAD, PAD + 2), (0, 0)))
    quad = jnp.concatenate(
        [vp[:, :, o:o + L] for o in (0, 1, W, W + 1)], axis=3
    )  # (n, HEADS, L, 4*hd)

    base = (jnp.clip(y0i, -1, H - 1) * W + jnp.clip(x0i, -1, W - 1) + PAD)
    idx = base.transpose(0, 2, 1, 3).reshape(n, HEADS, Lq * POINTS)
    g = jnp.take_along_axis(quad, idx[..., None], axis=2)
    samp = g.reshape(n, HEADS, Lq, POINTS, 4, hd)

    vx0 = ((x0i >= 0) & (x0i < W)).astype(x.dtype)
    vx1 = ((x0i + 1 >= 0) & (x0i + 1 < W)).astype(x.dtype)
    vy0 = ((y0i >= 0) & (y0i < H)).astype(x.dtype)
    vy1 = ((y0i + 1 >= 0) & (y0i + 1 < H)).astype(x.dtype)
    u0 = (1 - wx) * vx0
    u1 = wx * vx1
    t0 = (1 - wy) * vy0
    t1 = wy * vy1
    cw = jnp.stack([u0 * t0, u1 * t0, u0 * t1, u1 * t1], axis=-1)  # (n,Lq,HEADS,P,4)
    cw = cw * attw[..., None]
    out = jnp.einsum('nhlpcd,nlhpc->nlhd', samp, cw).reshape(n, Lq, c)
    a = out @ Wout + bout

    xr = xb + a
    h = _layernorm(xr, ln2_w, ln2_b)
    h = jax.nn.gelu(h @ W1 + b1, approximate=False)
    return xr + (h @ W2 + b2)


@functools.partial(jax.pmap, axis_name='i',
                   in_axes=(0, 0, 0) + (None,) * 16)
def _pmapped(x, ref, value, *weights):
    return _shard_fn(x, ref, value, *weights)


def kernel(**inputs) -> np.ndarray:
    x = np.asarray(inputs['x'], np.float32)
    ref = np.asarray(inputs['ref'], np.float32)
    value = np.asarray(inputs['value'], np.float32)
    wnames = ['ln1_w', 'ln1_b', 'ln2_w', 'ln2_b', 'Wv', 'bv', 'Woff', 'boff',
              'Wa', 'ba', 'Wout', 'bout', 'W1', 'b1', 'W2', 'b2']
    weights = [np.asarray(inputs[k], np.float32) for k in wnames]

    # Build shard stacks: shard s = (batch s//2, query half s%2).
    xs = np.stack([x[s // 2, (s % 2) * QSH:(s % 2 + 1) * QSH][None] for s in range(NSH)])
    refs = np.stack([ref[s // 2, (s % 2) * QSH:(s % 2 + 1) * QSH][None] for s in range(NSH)])
    vals = np.stack([value[s // 2][None] for s in range(NSH)])

    if jax.local_device_count() >= NSH:
        out = np.asarray(_pmapped(xs, refs, vals, *weights))  # (8, 1, QSH, C)
    else:  # robustness fallback: fewer devices visible -> sequential jit
        fn = jax.jit(_shard_fn)
        out = np.stack([np.asarray(fn(xs[s], refs[s], vals[s], *weights))
                        for s in range(NSH)])
    res = np.empty((N, LQ, C), np.float32)
    for s in range(NSH):
        res[s // 2, (s % 2) * QSH:(s % 2 + 1) * QSH] = out[s, 0]
    return res
